# revision 15
# baseline (speedup 1.0000x reference)
"""Trainium2 Bass kernel for nn_Lorec (moe_routing LoRA-with-soft-routing).

Computation (per batch b):
  gate_b = softmax(MLP(LayerNorm(ctr[b])))                    [16]
  A_b[i,r] = sum_j Wa[r*4096+i, j] gate_b[j]                  [4096,16]
  B_b[r,o] = sum_j Wb[r*4096+o, j] gate_b[j]                  [16,4096]
  out[b] = (x[b] @ A_b) @ B_b * 2.0                           [2048,4096]

Sharding: data-parallel over bs=8 across 8 NeuronCores (one batch per core).
Gating replicated on every core; each core selects its own batch's gate row
via a per-core one-hot input.

v2 (bf16): all bulk tensors are bf16 (x, Wa, Wb, A, B, xa, y) — halves HBM
traffic vs f32 (75 MB -> ~37 MB per core) and rides the full-rate bf16 PE
path. x is pre-transposed and tiled on the host into [sb*128+p, c*512+s]
layout so mm1 needs no on-device transposes (lhsT = A chunk, rhs = xT tile).
Gating scalars packed into one [128, 1074] f32 tensor -> single DMA.
G (I_16 kron gate) built arithmetically: c16[p] = gate[p%16] via one tiny
matmul with a constant selector, then G = mask * c16 — no scatter DMAs.
Output written as bf16 and upcast on host. SCALING folded into Wb on host.
"""

import sys

sys.path.insert(0, "/opt/trn_rl_repo")

import numpy as np
import ml_dtypes

BF = ml_dtypes.bfloat16

BS = 8
SEQ = 2048
IN = 4096
OUT = 4096
R = 16
CTR_OUT = 256
CTR_HID = 60
FD = 16  # FINAL_DIM
LN_EPS = 1e-5
SCALING = 2.0

P = 128
NSB = 4  # s-blocks per core
SBW = 512  # s-block width
NC_I = IN // P  # 32 i-chunks
NG = 8  # x DMA groups per s-block (4 chunks = 512 KB each; keeps PE-wait < HAM window)
GCH = NC_I // NG  # chunks per group
NOB = OUT // 512  # 8 o-blocks
N_WARM = 16  # junk PE warmup matmuls at t=0 (HAM un-throttle)

# gpk column layout (packed f32 gating constants; partition rows as noted)
C_W1T = 0  # [128, 120]
C_CTR = 120  # [8, 256]
C_GAM = 376  # [8, 256]
C_BET = 632  # [8, 256]
C_B1 = 888  # [60, 1]
C_W2T = 889  # [60, 16]
C_B2 = 905  # [16, 1]
C_T16 = 906  # [16, 128]  t16[j, p] = (p % 16 == j)
C_MSK = 1034  # [128, 32]  mask[p, h*16+r] = (h == r//8 and p//16 == r%8)
C_SEL = 1066  # [16, 8]   per-core one-hot columns
GPK_COLS = 1074

_COMPILED = None


def build_program():
    import concourse.mybir as mybir
    from concourse import bacc
    from concourse.masks import make_identity
    from concourse.tile import TileContext

    f32 = mybir.dt.float32
    bf16 = mybir.dt.bfloat16
    AX = mybir.AxisListType.X
    ALU = mybir.AluOpType
    ACTF = mybir.ActivationFunctionType

    nc = bacc.Bacc("TRN2", target_bir_lowering=False, debug=False, num_devices=BS)

    xtr_d = nc.dram_tensor("xtr", [NSB * P, NC_I * SBW], bf16, kind="ExternalInput").ap()
    gpk_d = nc.dram_tensor("gpk", [P, GPK_COLS], f32, kind="ExternalInput").ap()
    wap_d = nc.dram_tensor("wap", [P, 2 * IN], bf16, kind="ExternalInput").ap()
    wbp_d = nc.dram_tensor("wbp", [P, 2 * OUT], bf16, kind="ExternalInput").ap()
    y_d = nc.dram_tensor("y", [SEQ, OUT], bf16, kind="ExternalOutput").ap()

    with TileContext(nc) as tc:
        with (
            tc.tile_pool(name="const", bufs=1) as const,
            tc.tile_pool(name="gp", bufs=1) as gp,
            tc.tile_pool(name="wstream", bufs=4) as wstream,
            tc.tile_pool(name="xpool", bufs=24) as xpool,
            tc.tile_pool(name="xapool", bufs=2) as xapool,
            tc.tile_pool(name="opool", bufs=3) as opool,
            tc.tile_pool(name="psg_pool", bufs=1, space="PSUM") as psg_pool,
            tc.tile_pool(name="psA_pool", bufs=1, space="PSUM") as psA_pool,
            tc.tile_pool(name="psB_pool", bufs=2, space="PSUM") as psB_pool,
            tc.tile_pool(name="psxa_pool", bufs=1, space="PSUM") as psxa_pool,
            tc.tile_pool(name="psxb_pool", bufs=1, space="PSUM") as psxb_pool,
            tc.tile_pool(name="pso_pool", bufs=2, space="PSUM") as pso_pool,
        ):
            ident = const.tile([P, P], f32)
            make_identity(nc, ident)

            # ---- PE warm-up: junk matmuls while the first DMAs land, so the
            # HAM clock gate opens (1.2 -> 2.4 GHz) before real work arrives.
            for w in range(N_WARM):
                pwj = pso_pool.tile([P, 512], f32, tag="pso")
                nc.tensor.matmul(pwj[:, 0:P], ident[:], ident[:], start=True, stop=True)

            # ---- ACT table pre-warm: load every activation LUT used later
            # during the otherwise-idle engine-init window (each load is
            # ~1.3 us and would otherwise serialize inside the gating chain).
            wact = gp.tile([1, 4], f32)
            nc.gpsimd.memset(wact[:], 1.0)
            for fn in (ACTF.Identity, ACTF.Sqrt, ACTF.Relu, ACTF.Exp):
                nc.scalar.activation(wact[:], wact[:], fn)

            # ---- all HBM loads, queued in priority order on the sync ring ----
            gpk = gp.tile([P, GPK_COLS], f32)
            nc.sync.dma_start(out=gpk[:], in_=gpk_d[:])

            wa_t = []
            for h in range(2):
                wt = wstream.tile([P, IN], bf16, tag="wst")
                nc.sync.dma_start(out=wt[:], in_=wap_d[:, h * IN : (h + 1) * IN])
                wa_t.append(wt)
            wb_t = []
            for h in range(2):
                wt = wstream.tile([P, OUT], bf16, tag="wst")
                nc.sync.dma_start(out=wt[:], in_=wbp_d[:, h * OUT : (h + 1) * OUT])
                wb_t.append(wt)

            # ---- LayerNorm on ctr [8, 256] ----
            ctr = gpk[0:BS, C_CTR : C_CTR + CTR_OUT]
            gam = gpk[0:BS, C_GAM : C_GAM + CTR_OUT]
            bet = gpk[0:BS, C_BET : C_BET + CTR_OUT]
            mean = gp.tile([BS, 1], f32)
            xc = gp.tile([BS, CTR_OUT], f32)
            sq = gp.tile([BS, CTR_OUT], f32)
            vs = gp.tile([BS, 1], f32)
            std = gp.tile([BS, 1], f32)
            rstd = gp.tile([BS, 1], f32)
            hh = gp.tile([BS, CTR_OUT], f32)
            nc.vector.tensor_reduce(mean[:], ctr, axis=AX, op=ALU.add)
            nc.scalar.mul(mean[:], mean[:], 1.0 / CTR_OUT)
            nc.vector.tensor_scalar_sub(xc[:], ctr, mean[:])
            nc.vector.tensor_mul(sq[:], xc[:], xc[:])
            nc.vector.tensor_reduce(vs[:], sq[:], axis=AX, op=ALU.add)
            eps_t = gp.tile([BS, 1], f32)
            nc.gpsimd.memset(eps_t[:], LN_EPS)
            nc.scalar.activation(std[:], vs[:], ACTF.Sqrt, bias=eps_t[:], scale=1.0 / CTR_OUT)
            nc.vector.reciprocal(rstd[:], std[:])
            nc.vector.tensor_scalar_mul(hh[:], xc[:], rstd[:])
            nc.vector.tensor_mul(hh[:], hh[:], gam)
            nc.vector.tensor_add(hh[:], hh[:], bet)

            # ---- hT [256->2x128, 8] via PE transpose ----
            hT = gp.tile([P, 2 * BS], f32)
            for h in range(2):
                pt = psg_pool.tile([P, BS], f32, tag="psg")
                nc.tensor.transpose(pt[:], hh[:, h * P : (h + 1) * P], ident[0:BS, 0:BS])
                nc.scalar.copy(hT[:, h * BS : (h + 1) * BS], pt[:])

            # ---- h1T = relu(W1 @ h + b1) -> [60, 8] ----
            w1t = gpk[:, C_W1T : C_W1T + 2 * CTR_HID]
            ph1 = psg_pool.tile([CTR_HID, BS], f32, tag="psg")
            for h in range(2):
                nc.tensor.matmul(
                    ph1[:], w1t[:, h * CTR_HID : (h + 1) * CTR_HID],
                    hT[:, h * BS : (h + 1) * BS], start=(h == 0), stop=(h == 1),
                )
            h1T = gp.tile([CTR_HID, BS], f32)
            nc.scalar.activation(h1T[:], ph1[:], ACTF.Relu, bias=gpk[0:CTR_HID, C_B1 : C_B1 + 1])

            # ---- logitsT = W2 @ h1 + b2 -> [16, 8] ----
            plog = psg_pool.tile([FD, BS], f32, tag="psg")
            nc.tensor.matmul(plog[:], gpk[0:CTR_HID, C_W2T : C_W2T + FD], h1T[:], start=True, stop=True)
            logitsT = gp.tile([FD, BS], f32)
            nc.scalar.activation(logitsT[:], plog[:], ACTF.Identity, bias=gpk[0:FD, C_B2 : C_B2 + 1])

            # ---- softmax over FD per batch: transpose to [8, 16] ----
            plg = psg_pool.tile([BS, FD], f32, tag="psg")
            nc.tensor.transpose(plg[:], logitsT[:], ident[0:FD, 0:FD])
            lg = gp.tile([BS, FD], f32)
            nc.scalar.copy(lg[:], plg[:])
            mx = gp.tile([BS, 1], f32)
            ex = gp.tile([BS, FD], f32)
            sm = gp.tile([BS, 1], f32)
            rsm = gp.tile([BS, 1], f32)
            gate = gp.tile([BS, FD], f32)
            nc.vector.tensor_reduce(mx[:], lg[:], axis=AX, op=ALU.max)
            nc.vector.tensor_scalar_sub(ex[:], lg[:], mx[:])
            nc.scalar.activation(ex[:], ex[:], ACTF.Exp)
            nc.vector.tensor_reduce(sm[:], ex[:], axis=AX, op=ALU.add)
            nc.vector.reciprocal(rsm[:], sm[:])
            nc.vector.tensor_scalar_mul(gate[:], ex[:], rsm[:])

            # ---- gateT [16, 8], select own batch via one-hot columns ----
            pgT = psg_pool.tile([FD, BS], f32, tag="psg")
            nc.tensor.transpose(pgT[:], gate[:], ident[0:BS, 0:BS])
            gateT = gp.tile([FD, BS], f32)
            nc.scalar.copy(gateT[:], pgT[:])
            gsel = gp.tile([FD, BS], f32)
            gate_b = gp.tile([FD, 1], f32)
            nc.vector.tensor_mul(gsel[:], gateT[:], gpk[0:FD, C_SEL : C_SEL + BS])
            nc.vector.tensor_reduce(gate_b[:], gsel[:], axis=AX, op=ALU.add)

            # ---- G = I_16 kron gate_b, layout [128, 2*16], bf16 ----
            # c16[p] = gate_b[p % 16] via selector matmul, then G = mask * c16
            psc16 = psg_pool.tile([P, 1], f32, tag="psg")
            nc.tensor.matmul(
                psc16[:], gpk[0:FD, C_T16 : C_T16 + P], gate_b[:], start=True, stop=True
            )
            c16 = gp.tile([P, 1], f32)
            nc.scalar.copy(c16[:], psc16[:])
            Gf = gp.tile([P, 2 * FD], f32)
            nc.vector.tensor_scalar_mul(Gf[:], gpk[:, C_MSK : C_MSK + 2 * FD], c16[:])
            G = gp.tile([P, 2 * FD], bf16)
            nc.vector.tensor_copy(G[:], Gf[:])

            # ---- A-gen: A_sb[p, c*16+r] = A[c*128+p, r], bf16 ----
            A_sb = gp.tile([P, NC_I * R], bf16)
            psA = psA_pool.tile([P, 512], f32, tag="psA")
            for c in range(NC_I):
                for h in range(2):
                    nc.tensor.matmul(
                        psA[:, c * R : (c + 1) * R],
                        wa_t[h][:, c * P : (c + 1) * P],
                        G[:, h * FD : (h + 1) * FD],
                        start=(h == 0), stop=(h == 1),
                    )
            nc.scalar.copy(A_sb[:], psA[:])

            # ---- B-gen: B4 [128, 4096] bf16 — B at rows 0:16, replicated to
            # rows 32:48 / 64:80 / 96:112 (matching the col-tiled xa groups;
            # other rows are dead because the xaT lhsT rows there are zero).
            B4 = gp.tile([P, OUT], bf16)
            nc.gpsimd.memset(B4[:], 0.0)
            for ob in range(NOB):
                psB = psB_pool.tile([FD, 512], f32, tag="psB")
                for h in range(2):
                    nc.tensor.matmul(
                        psB[:],
                        G[:, h * FD : (h + 1) * FD],
                        wb_t[h][:, ob * 512 : (ob + 1) * 512],
                        start=(h == 0), stop=(h == 1),
                    )
                nc.vector.tensor_copy(B4[0:FD, ob * 512 : (ob + 1) * 512], psB[:])
            for j in range(1, 4):
                nc.gpsimd.dma_start(out=B4[32 * j : 32 * j + FD, :], in_=B4[0:FD, :])

            # persistent zero-padded xaT staging tiles (alternate per s-block)
            xaT_z = []
            for z in range(2):
                zt = gp.tile([P, SBW], bf16, name=f"xaTz{z}")
                nc.gpsimd.memset(zt[:], 0.0)
                xaT_z.append(zt)

            # ---- main loop over s-blocks (512 KB x-DMA groups) ----
            for sb in range(NSB):
                xts = []
                for g in range(NG):
                    xt = xpool.tile([P, GCH * SBW], bf16, tag="xg")
                    nc.sync.dma_start(
                        out=xt[:],
                        in_=xtr_d[sb * P : (sb + 1) * P, g * GCH * SBW : (g + 1) * GCH * SBW],
                    )
                    xts.append(xt)

                bank0 = psxa_pool.tile([P, SBW], f32, tag="psxa")
                bank1 = psxb_pool.tile([P, SBW], f32, tag="psxb")
                bank2 = psg_pool.tile([P, SBW], f32, tag="psg")
                bank3 = psA_pool.tile([P, SBW], f32, tag="psA")
                banks = [bank0, bank1, bank2, bank3]
                for c in range(NC_I):
                    j = c % 4
                    nc.tensor.matmul(
                        banks[j][32 * j : 32 * j + FD, :],
                        A_sb[:, c * R : (c + 1) * R],
                        xts[c // GCH][:, (c % GCH) * SBW : (c % GCH + 1) * SBW],
                        start=(c < 4), stop=(c >= NC_I - 4),
                        tile_position=(0, 32 * j),
                    )
                xaT = xaT_z[sb % 2]
                for j in range(4):
                    sl = slice(32 * j, 32 * j + FD)
                    if j % 2 == 0:
                        nc.scalar.copy(xaT[sl, :], banks[j][sl, :])
                    else:
                        nc.vector.tensor_copy(xaT[sl, :], banks[j][sl, :])

                for t in range(4):
                    out_sb = opool.tile([P, OUT], bf16, tag="osb")
                    for ob in range(NOB):
                        pso = pso_pool.tile([P, 512], f32, tag="pso")
                        nc.tensor.matmul(
                            pso[:],
                            xaT[:, t * P : (t + 1) * P],
                            B4[:, ob * 512 : (ob + 1) * 512],
                            start=True, stop=True,
                        )
                        if ob % 2 == 0:
                            nc.scalar.copy(out_sb[:, ob * 512 : (ob + 1) * 512], pso[:])
                        else:
                            nc.vector.tensor_copy(out_sb[:, ob * 512 : (ob + 1) * 512], pso[:])
                    nc.scalar.dma_start(
                        out=y_d[(sb * 4 + t) * P : (sb * 4 + t + 1) * P, :],
                        in_=out_sb[:],
                    )

    nc.compile()
    return nc


def host_prep(inputs):
    """Build per-core input arrays (layout + bf16 casts only, no math)."""
    x = np.asarray(inputs["x"], np.float32)

    gpk = np.zeros((P, GPK_COLS), np.float32)
    W1 = np.asarray(inputs["W1"], np.float32)
    gpk[:, C_W1T : C_W1T + 2 * CTR_HID] = (
        W1.T.reshape(2, P, CTR_HID).transpose(1, 0, 2).reshape(P, 2 * CTR_HID)
    )
    gpk[0:BS, C_CTR : C_CTR + CTR_OUT] = np.asarray(inputs["ctr_hidden_states"], np.float32)
    gpk[0:BS, C_GAM : C_GAM + CTR_OUT] = np.asarray(inputs["ln_gamma"], np.float32)[None, :]
    gpk[0:BS, C_BET : C_BET + CTR_OUT] = np.asarray(inputs["ln_beta"], np.float32)[None, :]
    gpk[0:CTR_HID, C_B1] = np.asarray(inputs["b1"], np.float32)
    gpk[0:CTR_HID, C_W2T : C_W2T + FD] = np.asarray(inputs["W2"], np.float32).T
    gpk[0:FD, C_B2] = np.asarray(inputs["b2"], np.float32)
    t16 = np.zeros((FD, P), np.float32)
    t16[np.arange(P) % FD, np.arange(P)] = 1.0
    gpk[0:FD, C_T16 : C_T16 + P] = t16
    mask = np.zeros((P, 2 * FD), np.float32)
    for r in range(FD):
        h, p0 = r // 8, (r % 8) * 16
        mask[p0 : p0 + FD, h * FD + r] = 1.0
    gpk[:, C_MSK : C_MSK + 2 * FD] = mask

    Wa = np.asarray(inputs["Wa"], np.float32)
    WaP = Wa.reshape(R, IN, FD).transpose(0, 2, 1).reshape(R * FD, IN)
    wap = np.ascontiguousarray(
        WaP.reshape(2, P, IN).transpose(1, 0, 2).reshape(P, 2 * IN)
    ).astype(BF)
    Wb = np.asarray(inputs["Wb"], np.float32) * SCALING
    WbP = Wb.reshape(R, OUT, FD).transpose(0, 2, 1).reshape(R * FD, OUT)
    wbp = np.ascontiguousarray(
        WbP.reshape(2, P, OUT).transpose(1, 0, 2).reshape(P, 2 * OUT)
    ).astype(BF)

    xbf = x.astype(BF)  # [8, 2048, 4096]

    in_maps = []
    for c in range(BS):
        g = gpk.copy()
        sel = np.zeros((FD, BS), np.float32)
        sel[:, c] = 1.0
        g[0:FD, C_SEL : C_SEL + BS] = sel
        # xtr[sb*128+p, cc*512+s] = x[c][sb*512+s, cc*128+p]
        xtr = np.ascontiguousarray(
            xbf[c].reshape(NSB, SBW, NC_I, P).transpose(0, 3, 2, 1)
        ).reshape(NSB * P, NC_I * SBW)
        in_maps.append({"gpk": g, "wap": wap, "wbp": wbp, "xtr": xtr})
    return in_maps


def get_compiled():
    global _COMPILED
    if _COMPILED is None:
        _COMPILED = build_program()
    return _COMPILED


def run(inputs, trace=False):
    from concourse.bass_utils import run_bass_kernel_spmd

    nc = get_compiled()
    in_maps = host_prep(inputs)
    res = run_bass_kernel_spmd(nc, in_maps, list(range(BS)), trace=trace)
    out = np.stack(
        [np.asarray(res.results[c]["y"]).astype(np.float32) for c in range(BS)], axis=0
    )
    return out, res


def kernel(**inputs) -> np.ndarray:
    out, _ = run(inputs, trace=False)
    return out


# revision 17
# speedup vs baseline: 1.0757x; 1.0757x over previous
"""Trainium2 Bass kernel for nn_Lorec (moe_routing LoRA-with-soft-routing).

Computation (per batch b):
  gate_b = softmax(MLP(LayerNorm(ctr[b])))                    [16]
  A_b[i,r] = sum_j Wa[r*4096+i, j] gate_b[j]                  [4096,16]
  B_b[r,o] = sum_j Wb[r*4096+o, j] gate_b[j]                  [16,4096]
  out[b] = (x[b] @ A_b) @ B_b * 2.0                           [2048,4096]

Sharding: data-parallel over bs=8 across 8 NeuronCores (one batch per core).
Gating replicated on every core; each core selects its own batch's gate row
via a per-core one-hot input.

v2 (bf16): all bulk tensors are bf16 (x, Wa, Wb, A, B, xa, y) — halves HBM
traffic vs f32 (75 MB -> ~37 MB per core) and rides the full-rate bf16 PE
path. x is pre-transposed and tiled on the host into [sb*128+p, c*512+s]
layout so mm1 needs no on-device transposes (lhsT = A chunk, rhs = xT tile).
Gating scalars packed into one [128, 1074] f32 tensor -> single DMA.
G (I_16 kron gate) built arithmetically: c16[p] = gate[p%16] via one tiny
matmul with a constant selector, then G = mask * c16 — no scatter DMAs.
Output written as bf16 and upcast on host. SCALING folded into Wb on host.
"""

import sys

sys.path.insert(0, "/opt/trn_rl_repo")

import numpy as np
import ml_dtypes

BF = ml_dtypes.bfloat16

BS = 8
SEQ = 2048
IN = 4096
OUT = 4096
R = 16
CTR_OUT = 256
CTR_HID = 60
FD = 16  # FINAL_DIM
LN_EPS = 1e-5
SCALING = 2.0

P = 128
NSB = 4  # s-blocks per core
SBW = 512  # s-block width
NC_I = IN // P  # 32 i-chunks
NG = 8  # x DMA groups per s-block (4 chunks = 512 KB each; keeps PE-wait < HAM window)
GCH = NC_I // NG  # chunks per group
NOB = OUT // 512  # 8 o-blocks
N_WARM = 16  # junk PE warmup matmuls at t=0 (HAM un-throttle)

# gpk column layout (packed f32 gating constants; partition rows as noted)
C_W1T = 0  # [128, 120]
C_CTR = 120  # [8, 256]
C_GAM = 376  # [8, 256]
C_BET = 632  # [8, 256]
C_B1 = 888  # [60, 1]
C_W2T = 889  # [60, 16]
C_B2 = 905  # [16, 1]
C_T16 = 906  # [16, 128]  t16[j, p] = (p % 16 == j)
C_MSK = 1034  # [128, 32]  mask[p, h*16+r] = (h == r//8 and p//16 == r%8)
C_SEL = 1066  # [16, 8]   per-core one-hot columns
GPK_COLS = 1074

_COMPILED = None


def build_program():
    import concourse.mybir as mybir
    from concourse import bacc
    from concourse.masks import make_identity
    from concourse.tile import TileContext

    f32 = mybir.dt.float32
    bf16 = mybir.dt.bfloat16
    AX = mybir.AxisListType.X
    ALU = mybir.AluOpType
    ACTF = mybir.ActivationFunctionType

    nc = bacc.Bacc("TRN2", target_bir_lowering=False, debug=False, num_devices=BS)

    xtr_d = nc.dram_tensor("xtr", [NSB * P, NC_I * SBW], bf16, kind="ExternalInput").ap()
    gpk_d = nc.dram_tensor("gpk", [P, GPK_COLS], f32, kind="ExternalInput").ap()
    wap_d = nc.dram_tensor("wap", [P, 2 * IN], bf16, kind="ExternalInput").ap()
    wbp_d = nc.dram_tensor("wbp", [P, 2 * OUT], bf16, kind="ExternalInput").ap()
    y_d = nc.dram_tensor("y", [SEQ, OUT], bf16, kind="ExternalOutput").ap()

    with TileContext(nc) as tc:
        with (
            tc.tile_pool(name="const", bufs=1) as const,
            tc.tile_pool(name="gp", bufs=1) as gp,
            tc.tile_pool(name="wstream", bufs=4) as wstream,
            tc.tile_pool(name="xpool", bufs=28) as xpool,
            tc.tile_pool(name="xapool", bufs=2) as xapool,
            tc.tile_pool(name="opool", bufs=3) as opool,
            tc.tile_pool(name="psg_pool", bufs=1, space="PSUM") as psg_pool,
            tc.tile_pool(name="psA_pool", bufs=1, space="PSUM") as psA_pool,
            tc.tile_pool(name="psB_pool", bufs=2, space="PSUM") as psB_pool,
            tc.tile_pool(name="psxa_pool", bufs=1, space="PSUM") as psxa_pool,
            tc.tile_pool(name="pso_pool", bufs=3, space="PSUM") as pso_pool,
        ):
            ident = const.tile([P, P], f32)
            make_identity(nc, ident)

            # ---- PE warm-up: junk matmuls while the first DMAs land, so the
            # HAM clock gate opens (1.2 -> 2.4 GHz) before real work arrives.
            for w in range(N_WARM):
                pwj = pso_pool.tile([P, 512], f32, tag="pso")
                nc.tensor.matmul(pwj[:, 0:P], ident[:], ident[:], start=True, stop=True)

            # ---- ACT table pre-warm: load every activation LUT used later
            # during the otherwise-idle engine-init window (each load is
            # ~1.3 us and would otherwise serialize inside the gating chain).
            wact = gp.tile([1, 4], f32)
            nc.gpsimd.memset(wact[:], 1.0)
            for fn in (ACTF.Identity, ACTF.Sqrt, ACTF.Relu, ACTF.Exp):
                nc.scalar.activation(wact[:], wact[:], fn)

            # ---- all HBM loads, queued in priority order on the sync ring ----
            gpk = gp.tile([P, GPK_COLS], f32)
            nc.sync.dma_start(out=gpk[:], in_=gpk_d[:])

            wa_t = []
            for h in range(2):
                wt = wstream.tile([P, IN], bf16, tag="wst")
                nc.sync.dma_start(out=wt[:], in_=wap_d[:, h * IN : (h + 1) * IN])
                wa_t.append(wt)
            wb_t = []
            for h in range(2):
                wt = wstream.tile([P, OUT], bf16, tag="wst")
                nc.sync.dma_start(out=wt[:], in_=wbp_d[:, h * OUT : (h + 1) * OUT])
                wb_t.append(wt)

            # ---- LayerNorm on ctr [8, 256] ----
            ctr = gpk[0:BS, C_CTR : C_CTR + CTR_OUT]
            gam = gpk[0:BS, C_GAM : C_GAM + CTR_OUT]
            bet = gpk[0:BS, C_BET : C_BET + CTR_OUT]
            mean = gp.tile([BS, 1], f32)
            xc = gp.tile([BS, CTR_OUT], f32)
            sq = gp.tile([BS, CTR_OUT], f32)
            vs = gp.tile([BS, 1], f32)
            std = gp.tile([BS, 1], f32)
            rstd = gp.tile([BS, 1], f32)
            hh = gp.tile([BS, CTR_OUT], f32)
            nc.vector.tensor_reduce(mean[:], ctr, axis=AX, op=ALU.add)
            nc.scalar.mul(mean[:], mean[:], 1.0 / CTR_OUT)
            nc.vector.tensor_scalar_sub(xc[:], ctr, mean[:])
            nc.vector.tensor_mul(sq[:], xc[:], xc[:])
            nc.vector.tensor_reduce(vs[:], sq[:], axis=AX, op=ALU.add)
            eps_t = gp.tile([BS, 1], f32)
            nc.gpsimd.memset(eps_t[:], LN_EPS)
            nc.scalar.activation(std[:], vs[:], ACTF.Sqrt, bias=eps_t[:], scale=1.0 / CTR_OUT)
            nc.vector.reciprocal(rstd[:], std[:])
            nc.vector.tensor_scalar_mul(hh[:], xc[:], rstd[:])
            nc.vector.tensor_mul(hh[:], hh[:], gam)
            nc.vector.tensor_add(hh[:], hh[:], bet)

            # ---- hT [256->2x128, 8] via PE transpose ----
            hT = gp.tile([P, 2 * BS], f32)
            for h in range(2):
                pt = psg_pool.tile([P, BS], f32, tag="psg")
                nc.tensor.transpose(pt[:], hh[:, h * P : (h + 1) * P], ident[0:BS, 0:BS])
                nc.scalar.copy(hT[:, h * BS : (h + 1) * BS], pt[:])

            # ---- h1T = relu(W1 @ h + b1) -> [60, 8] ----
            w1t = gpk[:, C_W1T : C_W1T + 2 * CTR_HID]
            ph1 = psg_pool.tile([CTR_HID, BS], f32, tag="psg")
            for h in range(2):
                nc.tensor.matmul(
                    ph1[:], w1t[:, h * CTR_HID : (h + 1) * CTR_HID],
                    hT[:, h * BS : (h + 1) * BS], start=(h == 0), stop=(h == 1),
                )
            h1T = gp.tile([CTR_HID, BS], f32)
            nc.scalar.activation(h1T[:], ph1[:], ACTF.Relu, bias=gpk[0:CTR_HID, C_B1 : C_B1 + 1])

            # ---- logitsT = W2 @ h1 + b2 -> [16, 8] ----
            plog = psg_pool.tile([FD, BS], f32, tag="psg")
            nc.tensor.matmul(plog[:], gpk[0:CTR_HID, C_W2T : C_W2T + FD], h1T[:], start=True, stop=True)
            logitsT = gp.tile([FD, BS], f32)
            nc.scalar.activation(logitsT[:], plog[:], ACTF.Identity, bias=gpk[0:FD, C_B2 : C_B2 + 1])

            # ---- softmax over FD per batch: transpose to [8, 16] ----
            plg = psg_pool.tile([BS, FD], f32, tag="psg")
            nc.tensor.transpose(plg[:], logitsT[:], ident[0:FD, 0:FD])
            lg = gp.tile([BS, FD], f32)
            nc.scalar.copy(lg[:], plg[:])
            mx = gp.tile([BS, 1], f32)
            ex = gp.tile([BS, FD], f32)
            sm = gp.tile([BS, 1], f32)
            rsm = gp.tile([BS, 1], f32)
            gate = gp.tile([BS, FD], f32)
            nc.vector.tensor_reduce(mx[:], lg[:], axis=AX, op=ALU.max)
            nc.vector.tensor_scalar_sub(ex[:], lg[:], mx[:])
            nc.scalar.activation(ex[:], ex[:], ACTF.Exp)
            nc.vector.tensor_reduce(sm[:], ex[:], axis=AX, op=ALU.add)
            nc.vector.reciprocal(rsm[:], sm[:])
            nc.vector.tensor_scalar_mul(gate[:], ex[:], rsm[:])

            # ---- gateT [16, 8], select own batch via one-hot columns ----
            pgT = psg_pool.tile([FD, BS], f32, tag="psg")
            nc.tensor.transpose(pgT[:], gate[:], ident[0:BS, 0:BS])
            gateT = gp.tile([FD, BS], f32)
            nc.scalar.copy(gateT[:], pgT[:])
            gsel = gp.tile([FD, BS], f32)
            gate_b = gp.tile([FD, 1], f32)
            nc.vector.tensor_mul(gsel[:], gateT[:], gpk[0:FD, C_SEL : C_SEL + BS])
            nc.vector.tensor_reduce(gate_b[:], gsel[:], axis=AX, op=ALU.add)

            # ---- G = I_16 kron gate_b, layout [128, 2*16], bf16 ----
            # c16[p] = gate_b[p % 16] via selector matmul, then G = mask * c16
            psc16 = psg_pool.tile([P, 1], f32, tag="psg")
            nc.tensor.matmul(
                psc16[:], gpk[0:FD, C_T16 : C_T16 + P], gate_b[:], start=True, stop=True
            )
            c16 = gp.tile([P, 1], f32)
            nc.scalar.copy(c16[:], psc16[:])
            Gf = gp.tile([P, 2 * FD], f32)
            nc.vector.tensor_scalar_mul(Gf[:], gpk[:, C_MSK : C_MSK + 2 * FD], c16[:])
            G = gp.tile([P, 2 * FD], bf16)
            nc.vector.tensor_copy(G[:], Gf[:])

            # ---- A-gen: A_sb[p, c*16+r] = A[c*128+p, r], bf16 ----
            A_sb = gp.tile([P, NC_I * R], bf16)
            psA = psA_pool.tile([P, 512], f32, tag="psA")
            for c in range(NC_I):
                for h in range(2):
                    nc.tensor.matmul(
                        psA[:, c * R : (c + 1) * R],
                        wa_t[h][:, c * P : (c + 1) * P],
                        G[:, h * FD : (h + 1) * FD],
                        start=(h == 0), stop=(h == 1),
                    )
            nc.scalar.copy(A_sb[:], psA[:])

            # ---- B-gen: B_sb [16, 4096] bf16 ----
            B_sb = gp.tile([FD, OUT], bf16)
            for ob in range(NOB):
                psB = psB_pool.tile([FD, 512], f32, tag="psB")
                for h in range(2):
                    nc.tensor.matmul(
                        psB[:],
                        G[:, h * FD : (h + 1) * FD],
                        wb_t[h][:, ob * 512 : (ob + 1) * 512],
                        start=(h == 0), stop=(h == 1),
                    )
                nc.vector.tensor_copy(B_sb[:, ob * 512 : (ob + 1) * 512], psB[:])

            def filler(n):
                """Junk matmuls into the (dead) psA bank: keep the PE busy while
                block 0's x groups stream in, so HAM enters the steady phase
                warm. Only used where the PE is data-starved anyway."""
                for _ in range(n):
                    pj = psA_pool.tile([P, 512], f32, tag="psA")
                    nc.tensor.matmul(
                        pj[0:FD, :], A_sb[:, 0:R], wa_t[0][:, 0:512],
                        start=True, stop=True,
                    )

            # ---- main loop over s-blocks (512 KB x-DMA groups) ----
            for sb in range(NSB):
                xts = []
                for g in range(NG):
                    xt = xpool.tile([P, GCH * SBW], bf16, tag="xg")
                    nc.sync.dma_start(
                        out=xt[:],
                        in_=xtr_d[sb * P : (sb + 1) * P, g * GCH * SBW : (g + 1) * GCH * SBW],
                    )
                    xts.append(xt)

                psxa = psxa_pool.tile([FD, SBW], f32, tag="psxa")
                for c in range(NC_I):
                    if sb == 0 and c % GCH == 0:
                        filler(8 if c == 0 else 2)
                    nc.tensor.matmul(
                        psxa[:],
                        A_sb[:, c * R : (c + 1) * R],
                        xts[c // GCH][:, (c % GCH) * SBW : (c % GCH + 1) * SBW],
                        start=(c == 0), stop=(c == NC_I - 1),
                    )
                xaT = xapool.tile([FD, SBW], bf16, tag="xaT")
                nc.vector.tensor_copy(xaT[:], psxa[:])

                for t in range(4):
                    out_sb = opool.tile([P, OUT], bf16, tag="osb")
                    for ob in range(NOB):
                        pso = pso_pool.tile([P, 512], f32, tag="pso")
                        nc.tensor.matmul(
                            pso[:],
                            xaT[:, t * P : (t + 1) * P],
                            B_sb[:, ob * 512 : (ob + 1) * 512],
                            start=True, stop=True,
                        )
                        if ob % 2 == 0:
                            nc.scalar.copy(out_sb[:, ob * 512 : (ob + 1) * 512], pso[:])
                        else:
                            nc.vector.tensor_copy(out_sb[:, ob * 512 : (ob + 1) * 512], pso[:])
                    if sb == NSB - 1:
                        # final block: write halves as soon as they are copied
                        nc.scalar.dma_start(
                            out=y_d[(sb * 4 + t) * P : (sb * 4 + t + 1) * P, 0 : OUT // 2],
                            in_=out_sb[:, 0 : OUT // 2],
                        )
                        nc.scalar.dma_start(
                            out=y_d[(sb * 4 + t) * P : (sb * 4 + t + 1) * P, OUT // 2 :],
                            in_=out_sb[:, OUT // 2 :],
                        )
                    else:
                        nc.scalar.dma_start(
                            out=y_d[(sb * 4 + t) * P : (sb * 4 + t + 1) * P, :],
                            in_=out_sb[:],
                        )

    nc.compile()
    return nc


def host_prep(inputs):
    """Build per-core input arrays (layout + bf16 casts only, no math)."""
    x = np.asarray(inputs["x"], np.float32)

    gpk = np.zeros((P, GPK_COLS), np.float32)
    W1 = np.asarray(inputs["W1"], np.float32)
    gpk[:, C_W1T : C_W1T + 2 * CTR_HID] = (
        W1.T.reshape(2, P, CTR_HID).transpose(1, 0, 2).reshape(P, 2 * CTR_HID)
    )
    gpk[0:BS, C_CTR : C_CTR + CTR_OUT] = np.asarray(inputs["ctr_hidden_states"], np.float32)
    gpk[0:BS, C_GAM : C_GAM + CTR_OUT] = np.asarray(inputs["ln_gamma"], np.float32)[None, :]
    gpk[0:BS, C_BET : C_BET + CTR_OUT] = np.asarray(inputs["ln_beta"], np.float32)[None, :]
    gpk[0:CTR_HID, C_B1] = np.asarray(inputs["b1"], np.float32)
    gpk[0:CTR_HID, C_W2T : C_W2T + FD] = np.asarray(inputs["W2"], np.float32).T
    gpk[0:FD, C_B2] = np.asarray(inputs["b2"], np.float32)
    t16 = np.zeros((FD, P), np.float32)
    t16[np.arange(P) % FD, np.arange(P)] = 1.0
    gpk[0:FD, C_T16 : C_T16 + P] = t16
    mask = np.zeros((P, 2 * FD), np.float32)
    for r in range(FD):
        h, p0 = r // 8, (r % 8) * 16
        mask[p0 : p0 + FD, h * FD + r] = 1.0
    gpk[:, C_MSK : C_MSK + 2 * FD] = mask

    Wa = np.asarray(inputs["Wa"], np.float32)
    WaP = Wa.reshape(R, IN, FD).transpose(0, 2, 1).reshape(R * FD, IN)
    wap = np.ascontiguousarray(
        WaP.reshape(2, P, IN).transpose(1, 0, 2).reshape(P, 2 * IN)
    ).astype(BF)
    Wb = np.asarray(inputs["Wb"], np.float32) * SCALING
    WbP = Wb.reshape(R, OUT, FD).transpose(0, 2, 1).reshape(R * FD, OUT)
    wbp = np.ascontiguousarray(
        WbP.reshape(2, P, OUT).transpose(1, 0, 2).reshape(P, 2 * OUT)
    ).astype(BF)

    xbf = x.astype(BF)  # [8, 2048, 4096]

    in_maps = []
    for c in range(BS):
        g = gpk.copy()
        sel = np.zeros((FD, BS), np.float32)
        sel[:, c] = 1.0
        g[0:FD, C_SEL : C_SEL + BS] = sel
        # xtr[sb*128+p, cc*512+s] = x[c][sb*512+s, cc*128+p]
        xtr = np.ascontiguousarray(
            xbf[c].reshape(NSB, SBW, NC_I, P).transpose(0, 3, 2, 1)
        ).reshape(NSB * P, NC_I * SBW)
        in_maps.append({"gpk": g, "wap": wap, "wbp": wbp, "xtr": xtr})
    return in_maps


def get_compiled():
    global _COMPILED
    if _COMPILED is None:
        _COMPILED = build_program()
    return _COMPILED


def run(inputs, trace=False):
    from concourse.bass_utils import run_bass_kernel_spmd

    nc = get_compiled()
    in_maps = host_prep(inputs)
    res = run_bass_kernel_spmd(nc, in_maps, list(range(BS)), trace=trace)
    out = np.stack(
        [np.asarray(res.results[c]["y"]).astype(np.float32) for c in range(BS)], axis=0
    )
    return out, res


def kernel(**inputs) -> np.ndarray:
    out, _ = run(inputs, trace=False)
    return out
